# revision 43
# baseline (speedup 1.0000x reference)
"""Trainium2 Bass kernel for batched per-sample expert matmul (MoE routing).

Computes y[n, i] = relu(b[idxs[n], i] + sum_o w[idxs[n], i, o] * x[n, o])
for x (8192, 256), idxs (8192,), w (64, 256, 256), b (64, 256).

Strategy (expert-parallel, bf16 traffic)
-----------------------------------------
The whole problem is DMA-wire bound (~200 GB/s sustained per core, one
queue at a time; concurrent rings share the fabric round-robin and lose
aggregate bandwidth), so the kernel is shaped around minimizing HBM
bytes and the post-stream tail:

Host side (numpy, cheap):
  * Route tokens by expert and assign each expert to exactly ONE core
    (greedy balance on token counts) -> every weight matrix crosses HBM
    exactly once, on one core: 1.0 MB/core of bf16 weights instead of
    the 16 MB replicated table.
  * All matmul traffic is cast to bf16 (inputs, weights, outputs). The
    error budget (absmax-relative 2e-2) leaves ~6x margin over bf16's
    ~3e-3.
  * Each core gets NSLOT expert slots; slot j is padded to a uniform
    per-rank token capacity cap[j] (slots sorted by count, so rank
    capacities hug the actual distribution).  One DRAM input tensor per
    core packs [w | xT] per slot in consumption order.

Device side (one static Tile program, identical on all 8 cores):
  Output features live on PSUM partitions, tokens on the free dim, so
  one expert = one PSUM bank and the bias is a per-partition scalar:
    ps[:, 0:C]       = wT00 @ x0 + wT10 @ x1      (i-chunk 0)
    ps[:, 256:256+C] = wT01 @ x0 + wT11 @ x1      (i-chunk 1)
    y0 = max(ps[:, 0:C] + bias0, 0)      (DVE, per-partition bias)
    y1 = relu(ps[:, 256:..] * 1 + bias1) (ACT, runs parallel to DVE)
  No bias matmuls, no weight duplication; 8 slots = 8 PSUM banks, so
  one accumulation group per bank (start=True zeroes the WHOLE bank)
  and no PSUM reuse waits.  The in-stream rides the single SWDGE
  (gpsimd) queue in small need-order batches (head/tail slots alone,
  pairs in the middle) so each slot's data plus its ~0.9us completion
  semaphore land as early as possible.  Outputs drain in three waves
  chasing the compute, split across the Sync ring (DVE-drained halves)
  and the then-idle gpsimd queue (ACT halves) so issue costs overlap.
  Dummy DVE/ACT ops absorb the bias-DMA wait so every matmul and drain
  carries at most one semaphore wait (walrus constraint).

Host side: scatter the per-slot token blocks back to sample order.
Pathological expert skew (>256 tokens per chunk slot or >8 slots per
core) falls back to extra passes of the same program shape.

Rejected by experiment (all correct but slower): fp32r replicated-table
baseline (34us), 2-3 concurrent in-rings (25-31us), int8 weights via
casting SWDGE DMA (the wire is destination-byte paced) or via DVE/ACT
widening ops (convert latency eats the savings), per-slot in-batches
(SWDGE descriptor-generation overhead), fp8 anywhere (blows the 2e-2
error budget).
"""

import os

import numpy as np

import concourse.bacc as bacc
import concourse.bass as bass
import concourse.mybir as mybir
import concourse.tile as tile
from concourse.bass_utils import run_bass_kernel_spmd

try:
    import ml_dtypes

    BF16 = ml_dtypes.bfloat16
except ImportError:  # pragma: no cover
    BF16 = np.dtype("bfloat16")

N_CORES = 8
P = 128
F = 256
CAP_MAX = 256    # tokens per slot (2 i-chunks of <=256 f32 fill one PSUM bank)
NSLOT = 8        # expert slots per core per pass == PSUM banks

# Set by the last kernel() call when KBENCH_TRACE=1 (used by test.py only).
LAST_EXEC_TIME_NS = None
LAST_TRACE = None

_PROGRAM_CACHE = {}


def _build_schedule(idxs: np.ndarray, n_experts: int):
    """Assign expert chunks (<=CAP_MAX tokens) to cores, balanced by count.

    Returns a list of scheduling passes; each pass is a list of per-core
    slot lists [(expert, tok_array), ...] sorted by descending token count.
    """
    toks_by_e = [np.nonzero(idxs == e)[0] for e in range(n_experts)]
    chunks = []
    for e, toks in enumerate(toks_by_e):
        for k in range(0, len(toks), CAP_MAX):
            chunks.append((e, toks[k:k + CAP_MAX]))
    chunks.sort(key=lambda c: -len(c[1]))

    npass = max(1, -(-len(chunks) // (N_CORES * NSLOT)))
    cores = [[] for _ in range(N_CORES * npass)]
    load = [0] * (N_CORES * npass)
    for e, toks in chunks:
        cand = min(
            (i for i in range(len(cores)) if len(cores[i]) < NSLOT),
            key=lambda i: load[i],
        )
        cores[cand].append((e, toks))
        load[cand] += len(toks)
    for sl in cores:
        sl.sort(key=lambda c: -len(c[1]))
    return [cores[p * N_CORES:(p + 1) * N_CORES] for p in range(npass)]


def _build_program(caps: tuple):
    nslot = len(caps)
    S = sum(caps)
    xoff = [0]
    for c in caps:
        xoff.append(xoff[-1] + c)

    int8_w = os.environ.get("KINT8", "0") == "1"

    nc = bacc.Bacc(
        "TRN2", target_bir_lowering=False, debug=False, num_devices=N_CORES
    )
    bf16 = mybir.dt.bfloat16
    f32 = mybir.dt.float32
    bias_d = nc.dram_tensor("bias", [P, 2 * nslot], f32, kind="ExternalInput").ap()
    y_d = nc.dram_tensor("y", [P, 2 * S], bf16, kind="ExternalOutput").ap()

    relu = mybir.ActivationFunctionType.Relu
    add = mybir.AluOpType.add
    amax = mybir.AluOpType.max

    # in-stream batches over whole slots: small head batch to prime the
    # pipeline.  The int8 path pays two DMAs (w, x) per batch, so it uses
    # fewer, bigger batches to keep SWDGE descriptor generation (~0.7us per
    # DMA) below the wire time.
    if nslot <= 2:
        groups = [[j] for j in range(nslot)]
    elif int8_w:
        mid = (nslot - 1) // 2
        groups = [[0], list(range(1, mid + 1)), list(range(mid + 1, nslot))]
        groups = [g for g in groups if g]
    else:
        groups = [[0]]
        mid = list(range(1, nslot - 1))
        k = int(os.environ.get("KBAT", "2"))
        groups += [mid[i:i + k] for i in range(0, len(mid), k)]
        groups += [[nslot - 1]]

    inq = nc.gpsimd
    outq = nc.sync

    with tile.TileContext(nc) as tc:
        with (
            tc.tile_pool(name="const", bufs=1) as const,
            tc.tile_pool(name="inb", bufs=1) as inpool,
            tc.tile_pool(name="yout", bufs=1) as ypool,
            tc.tile_pool(name="ps", bufs=8, space="PSUM") as pspool,
        ):
            bt = const.tile([P, 2 * nslot], f32, tag="bias")
            bias_emitted = False

            tiles = {}
            xtiles = {}
            xheads = {}
            if int8_w:
                # w ships as int8 (values in [-127,127], exact in bf16;
                # per-contraction-row quant scales are folded into x on the
                # host) and is widened to bf16 on whichever of DVE/ACT is
                # free, halving the weight wire bytes.  w and x alternate on
                # the one gpsimd queue in need order.
                w_d = nc.dram_tensor(
                    "wt", [P, nslot * 4 * P], mybir.dt.int8, kind="ExternalInput"
                ).ap()
                x_d = nc.dram_tensor(
                    "xt", [P, 2 * S], bf16, kind="ExternalInput"
                ).ap()
                for g, slots in enumerate(groups):
                    lo, hi = slots[0], slots[-1] + 1
                    t = inpool.tile(
                        [P, (hi - lo) * 4 * P], mybir.dt.int8, tag=f"w{g}"
                    )
                    for j in slots:
                        tiles[j] = (t, (j - lo) * 4 * P)
                    inq.dma_start(t[:], w_d[:, lo * 4 * P:hi * 4 * P])
                    xt = inpool.tile(
                        [P, 2 * (xoff[hi] - xoff[lo])], bf16, tag=f"x{g}"
                    )
                    for j in slots:
                        xtiles[j] = (xt, 2 * (xoff[j] - xoff[lo]))
                    xheads[lo] = xt
                    inq.dma_start(xt[:], x_d[:, 2 * xoff[lo]:2 * xoff[hi]])
            else:
                NCOL = nslot * 4 * P + 2 * S
                in_d = nc.dram_tensor(
                    "inp", [P, NCOL], bf16, kind="ExternalInput"
                ).ap()

                def slot_col(j):
                    return j * 4 * P + 2 * xoff[j]

                # The head batch rides the Sync ring, which starts ~0.7us
                # earlier than gpsimd out of the preamble; everything else
                # stays on the one SWDGE queue (concurrent rings share the
                # DMA fabric round-robin and lose aggregate bandwidth).
                head_sync = os.environ.get("KHEAD", "0") == "1"
                groups = [
                    (outq if head_sync and g == 0 and len(groups) > 1 else inq,
                     sl)
                    for g, sl in enumerate(groups)
                ]

                for g, (q, slots) in enumerate(groups):
                    lo, hi = slots[0], slots[-1] + 1
                    a, b = slot_col(lo), slot_col(hi) if hi < nslot else NCOL
                    t = inpool.tile([P, b - a], bf16, tag=f"in{g}")
                    for j in slots:
                        tiles[j] = (t, slot_col(j) - a)
                        xtiles[j] = (t, slot_col(j) - a + 4 * P)
                    q.dma_start(t[:], in_d[:, a:b])
                    if q is outq and not bias_emitted:
                        # bias follows the head batch on sync: off the
                        # critical path, needed only by the first drain.
                        outq.dma_start(bt[:], bias_d[:])
                        bias_emitted = True
            if not bias_emitted:
                outq.dma_start(bt[:], bias_d[:])

            wconv = {}
            if int8_w:
                wcpool = inpool

                def wv(j, c0, c1):
                    o = (c0 * 2 + c1) * P
                    return wconv[j][:, o:o + P]
            else:

                def wv(j, c0, c1):
                    t, base = tiles[j]
                    o = base + (c0 * 2 + c1) * P
                    return t[:, o:o + P]

            def xv(j, c0):
                t, base = xtiles[j]
                o = base + c0 * caps[j]
                return t[:, o:o + caps[j]]

            yt0 = ypool.tile([P, S], bf16, tag="y0")
            yt1 = ypool.tile([P, S], bf16, tag="y1")
            scr0 = const.tile([P, 1], f32, tag="scr0")
            scr1 = const.tile([P, 1], f32, tag="scr1")
            # Absorb the bias-DMA wait (and ACT's one-time relu table load)
            # off the critical path so the real drains carry only the PE
            # semaphore wait.
            nc.vector.tensor_scalar(scr0[:], bt[:, 0:1], 0.0, None, add)
            nc.scalar.activation(scr1[:], bt[:, 0:1], relu)

            owave = [0]
            # Output waves: last wave is the (smallest) final slot alone so
            # the post-stream tail is minimal; earlier waves flush every two
            # slots and their wire overlaps the in-stream (DRAM reads and
            # writes share the fabric only partially).
            flush_at = {nslot - 1}
            kd = (1, 3, 5) if os.environ.get("KWD", "135") == "135" else (2, 4, 6)
            for d in kd:
                if nslot > d:
                    flush_at.add(nslot - 1 - d)
            copy = mybir.ActivationFunctionType.Copy
            for j in range(nslot):
                C = caps[j]
                ps = pspool.tile([P, 2 * F], f32)
                if int8_w:
                    # Widen this slot's int8 weights to bf16, alternating
                    # engines so neither becomes the bottleneck.
                    t, base = tiles[j]
                    wc = wcpool.tile([P, 4 * P], bf16, tag=f"wc{j}")
                    wconv[j] = wc
                    if j % 2 == 0:
                        nc.vector.tensor_scalar(
                            wc[:], t[:, base:base + 4 * P], 0.0, None, add
                        )
                    else:
                        nc.scalar.activation(wc[:], t[:, base:base + 4 * P], copy)
                    if j in xheads:
                        # Dummy matmul absorbs the x-batch DMA wait into PE
                        # program order; the real group's start=True re-zeroes
                        # the bank afterwards.
                        xb = xheads[j]
                        nc.tensor.matmul(
                            ps[0:2, 508:510], xb[:, 0:2], xb[:, 0:2],
                            start=True, stop=True,
                        )
                # One accumulation group per PSUM bank: start=True zeroes the
                # WHOLE bank, so only the first matmul opens it and only the
                # last one closes it.
                nc.tensor.matmul(
                    ps[:, 0:C], wv(j, 0, 0), xv(j, 0), start=True, stop=False
                )
                nc.tensor.matmul(
                    ps[:, F:F + C], wv(j, 0, 1), xv(j, 0), start=False, stop=False
                )
                nc.tensor.matmul(
                    ps[:, 0:C], wv(j, 1, 0), xv(j, 1), start=False, stop=False
                )
                nc.tensor.matmul(
                    ps[:, F:F + C], wv(j, 1, 1), xv(j, 1), start=False, stop=True
                )
                o = xoff[j]
                nc.vector.tensor_scalar(
                    yt0[:, o:o + C], ps[:, 0:C], bt[:, 2 * j:2 * j + 1], 0.0,
                    add, amax,
                )
                nc.scalar.activation(
                    yt1[:, o:o + C], ps[:, F:F + C], relu,
                    bias=bt[:, 2 * j + 1:2 * j + 2],
                )
                # Flush outputs in waves; the last wave is just the smallest
                # slot so the post-stream tail stays short.
                if j in flush_at:
                    # yt0 flushes ride sync, yt1 flushes ride the gpsimd
                    # queue (idle once the in-stream is done) so the issue
                    # costs overlap instead of serializing on one ring.
                    lo, hi = owave[0], xoff[j + 1]
                    owave = [hi]
                    outq.dma_start(y_d[:, lo:hi], yt0[:, lo:hi])
                    inq.dma_start(y_d[:, S + lo:S + hi], yt1[:, lo:hi])
    nc.compile()
    return nc


def kernel(x: np.ndarray, idxs: np.ndarray, w: np.ndarray, b: np.ndarray) -> np.ndarray:
    global LAST_EXEC_TIME_NS, LAST_TRACE
    x = np.ascontiguousarray(x, dtype=np.float32)
    w = np.ascontiguousarray(w, dtype=np.float32)
    b = np.ascontiguousarray(b, dtype=np.float32)
    idxs_np = np.asarray(idxs).astype(np.int64)
    B = x.shape[0]

    int8_w = os.environ.get("KINT8", "0") == "1"
    if int8_w:
        # Symmetric int8 per (expert, contraction row) o; scales are folded
        # into x per slot on the host, so the device sees plain bf16 math.
        wscale = np.abs(w).max(axis=1) / 127.0          # (64, 256)
        wscale = np.maximum(wscale, 1e-30)
        wq = np.round(w / wscale[:, None, :]).clip(-127, 127).astype(np.int8)
        wblk = np.ascontiguousarray(
            wq.reshape(-1, 2, P, 2, P)     # (e, c1, m, c0, p)
            .transpose(0, 4, 3, 1, 2)      # (e, p, c0, c1, m)
            .reshape(-1, P, 4 * P)
        )
    else:
        x16 = x.astype(BF16)
        # Per-expert weight blocks in PE layout:
        # wblk[e, p, (c0*2+c1)*128 + m] = w[e, c1*128+m, c0*128+p]
        wblk = np.ascontiguousarray(
            w.reshape(-1, 2, P, 2, P)          # (e, c1, m, c0, p)
            .transpose(0, 4, 3, 1, 2)          # (e, p, c0, c1, m)
            .reshape(-1, P, 4 * P)
            .astype(BF16)
        )

    passes = _build_schedule(idxs_np, w.shape[0])
    trace = bool(os.environ.get("KBENCH_TRACE"))
    y = np.empty((B, F), dtype=np.float32)

    for cores in passes:
        nslot = max(1, max(len(sl) for sl in cores))
        caps = tuple(
            max(4, -4 * (-max(
                (len(sl[j][1]) if j < len(sl) else 0) for sl in cores
            ) // 4))
            for j in range(nslot)
        )
        S = sum(caps)
        xoff = np.concatenate([[0], np.cumsum(caps)]).astype(int)
        NCOL = nslot * 4 * P + 2 * S

        key = caps
        if key not in _PROGRAM_CACHE:
            _PROGRAM_CACHE[key] = _build_program(caps)
        nc = _PROGRAM_CACHE[key]

        in_maps = []
        for sl in cores:
            bias = np.zeros((P, 2 * nslot), dtype=np.float32)
            if int8_w:
                wt = np.zeros((P, nslot * 4 * P), dtype=np.int8)
                xt_full = np.zeros((P, 2 * S), dtype=BF16)
            else:
                inp = np.zeros((P, NCOL), dtype=BF16)
            for j, (e, toks) in enumerate(sl):
                n = len(toks)
                if int8_w:
                    wt[:, j * 4 * P:(j + 1) * 4 * P] = wblk[e]
                    xs = (x[toks] * wscale[e]).astype(BF16)
                    xt = xs.T.reshape(2, P, n).transpose(1, 0, 2)
                    xcols = xt_full[:, 2 * xoff[j]:2 * xoff[j + 1]].reshape(
                        P, 2, caps[j]
                    )
                else:
                    col = j * 4 * P + 2 * xoff[j]
                    inp[:, col:col + 4 * P] = wblk[e]
                    # xT[p, c0, t] = x[tok_t, c0*128 + p]
                    xt = x16[toks].T.reshape(2, P, n).transpose(1, 0, 2)
                    xcols = inp[:, col + 4 * P:col + 4 * P + 2 * caps[j]].reshape(
                        P, 2, caps[j]
                    )
                xcols[:, :, :n] = xt
                bias[:, 2 * j] = b[e, 0:P]
                bias[:, 2 * j + 1] = b[e, P:2 * P]
            if int8_w:
                in_maps.append({"wt": wt, "xt": xt_full, "bias": bias})
            else:
                in_maps.append({"inp": inp, "bias": bias})

        res = run_bass_kernel_spmd(
            nc, in_maps, core_ids=list(range(N_CORES)), trace=trace
        )
        LAST_EXEC_TIME_NS = res.exec_time_ns
        LAST_TRACE = res.instructions_and_trace

        for c, sl in enumerate(cores):
            yc = np.asarray(res.results[c]["y"]).reshape(P, 2, S)
            for j, (e, toks) in enumerate(sl):
                n = len(toks)
                o = xoff[j]
                y[toks] = (
                    yc[:, :, o:o + n].transpose(2, 1, 0).reshape(n, F)
                )
    return y
